# revision 10
# baseline (speedup 1.0000x reference)
"""Causal attention (QKV proj + softmax + PV + ReLU) on 8 trn2 NeuronCores.

Sharding: data-parallel over batch B=32 -> 4 batches per core; projection
weights replicated.

Dtype strategy (error budget is max-abs-err / global absmax < 2e-2; host
numpy simulation of the exact quantization pipeline predicts 1.58e-2):
  * All projections and most of the attention run as fp8(e4m3) DoubleRow
    matmuls: the PE streams 1 output column/cycle regardless of dtype,
    but DoubleRow packs TWO 128-row contraction blocks per matmul
    (APs [128, 2, cols]), halving matmul count for the C=1024 (proj) and
    D=512 (S) contractions.
  * The output's first 128 rows attend to <=128 keys, so softmax weight
    concentrates there and fp8 noise on P/V/logits would blow the max-err
    budget. Those rows get a small fp16 path: fp16 V[0:128] (from an fp16
    copy of x[:, :, :128] + fp16 wv), an fp16 (i<128, j<128) logit block
    (from fp16-stored q/k), fp16 P, fp16 PV for the ib=0 row tile. For
    i>=128 softmax weight is spread (<~3% each), so fp8-quantized q/k
    storage, P, and V noise all average out.
  * fp8 scaling: W rows ~U(-1/32,1/32) sit below e4m3's min normal 2^-6,
    so wq/wk are host-prescaled x128 (and wv x32). Q/K drain at x1/32
    (fp8 stores 4Q; 16x folds into the exp scale) and x1/128 (fp16 corner
    copies). V drains at +32bv into fp8 (32V); the 32 folds into the
    fp8 rowsum via ones8=32 (fp16 path: plain V, ones16=1).

Per core, per batch:
  Q^T,K^T[d,l]: 4 DoubleRow MMs per [d-block, l-chunk]; Q drains on ACT
      (activation Identity, per-partition bias), K on DVE - splits the
      drain load across engines. Stored as fp8 pair tiles [128,2,L]
      (dt pairs, ready as DoubleRow operands) + tiny fp16 l<128 copies.
  V[l,d]: 4 DoubleRow MMs per l-tile -> fp8 pair tiles [128,2,512]
      (j-parity pairs for the PV DoubleRow contraction); plus fp16
      V[0:128] from the fp16 x/wv copies.
  S^T[j,i]: 2 DoubleRow MMs per causal chunk (d pairs); the (i<128,j<128)
      corner chunk runs fp16. exp on ACT (scale SCALE/16 fp8 / SCALE
      fp16, per-partition padmask bias) drains P^T straight to fp8 pair
      tiles [128,2,L] (j-parity); diagonal tiles masked by DVE multiply
      with tri; gap regions of odd-j tiles memset to 0 so even-ib
      DoubleRow reads see zeros.
  O' = P^T.T @ V as DoubleRow over j-pairs (ib=0: fp16); rowsum
      piggybacks on the stationary (moving ones8 pair / ones16).
  out = Relu(O' * (1/rowsum)): reciprocal on DVE, normalize+relu on
      GpSimd (keeps DVE off the PV critical path), DMA out.

DMA-queue/warmup notes: x prefetch on nc.sync, const loads on nc.scalar,
stores on nc.gpsimd (SWDGE) so they never head-of-line-block the
prefetch; dummy warmup matmuls pre-warm the PE HAM clock-gate to 2.4 GHz
while batch-0 inputs stream in; wk/wv loads deferred behind batch-0 x.
"""

import os
from contextlib import ExitStack

import numpy as np
import ml_dtypes

import concourse.tile as tile
from concourse import bacc, mybir
from concourse import bass_utils

F32 = mybir.dt.float32
F16 = mybir.dt.float16
F8 = mybir.dt.float8e4
DR = mybir.MatmulPerfMode.DoubleRow
AF = mybir.ActivationFunctionType
ALU = mybir.AluOpType

N_CORES = 8
B = 32
L = 1024
C = 1024  # d_model
D = 512
P = 128
NB = B // N_CORES  # batches per core
CT = C // P  # 8 contraction tiles
CTP = CT // 2  # 4 DoubleRow contraction pairs
DT = D // P  # 4 d tiles
DTP = DT // 2  # 2 DoubleRow d pairs
LT = L // P  # 8 l/j/i tiles
LTP = LT // 2  # 4 j-pair tiles
SCALE = float(D) ** -0.5
NEG = -30000.0
WS = 128.0  # host prescale on wq/wk
VS = 32.0  # host prescale on wv (fp8 V stored as 32V)
QKS = 4.0  # fp8 q/k stored as 4Q
EXP_SCALE8 = SCALE / (QKS * QKS)


def build_program(nb: int = NB):
    """Build the per-core Bass program for nb batches."""
    nc = bacc.Bacc("TRN2", target_bir_lowering=False, debug=False,
                   num_devices=N_CORES)

    xtb = nc.dram_tensor("xtb", [nb, P, CT, L], F8, kind="ExternalInput").ap()
    xtb16 = nc.dram_tensor("xtb16", [nb, P, CT, P], F16,
                           kind="ExternalInput").ap()
    wqT = nc.dram_tensor("wqT", [C, D], F8, kind="ExternalInput").ap()
    wkT = nc.dram_tensor("wkT", [C, D], F8, kind="ExternalInput").ap()
    wvT = nc.dram_tensor("wvT", [C, D], F8, kind="ExternalInput").ap()
    wvT16 = nc.dram_tensor("wvT16", [C, D], F16, kind="ExternalInput").ap()
    bq4 = nc.dram_tensor("bq4", [P, DT], F32, kind="ExternalInput").ap()
    bq1 = nc.dram_tensor("bq1", [P, DT], F32, kind="ExternalInput").ap()
    bk128 = nc.dram_tensor("bk128", [P, DT], F32, kind="ExternalInput").ap()
    bv32 = nc.dram_tensor("bv32", [P, D], F32, kind="ExternalInput").ap()
    bv1 = nc.dram_tensor("bv1", [P, D], F32, kind="ExternalInput").ap()
    pmt = nc.dram_tensor("pmt", [nb, P, LT], F32, kind="ExternalInput").ap()
    tri = nc.dram_tensor("tri", [P, P], F16, kind="ExternalInput").ap()
    out = nc.dram_tensor("out", [nb, L, D], F32, kind="ExternalOutput").ap()

    with tile.TileContext(nc) as tc, ExitStack() as ctx:
        const = ctx.enter_context(tc.tile_pool(name="const", bufs=1))
        xt_pool = ctx.enter_context(tc.tile_pool(name="xt", bufs=3))
        qk_pool = ctx.enter_context(tc.tile_pool(name="qk", bufs=2))
        v_pool = ctx.enter_context(tc.tile_pool(name="v", bufs=2))
        pt_pool = ctx.enter_context(tc.tile_pool(name="pt", bufs=2))
        o_pool = ctx.enter_context(tc.tile_pool(name="o", bufs=3))
        sm_pool = ctx.enter_context(tc.tile_pool(name="sm", bufs=4))
        pm_pool = ctx.enter_context(tc.tile_pool(name="pm", bufs=2))
        proj_ps = ctx.enter_context(tc.tile_pool(name="pps", bufs=3, space="PSUM"))
        s_ps = ctx.enter_context(tc.tile_pool(name="sps", bufs=2, space="PSUM"))
        o_ps = ctx.enter_context(tc.tile_pool(name="ops", bufs=2, space="PSUM"))
        r_ps = ctx.enter_context(tc.tile_pool(name="rps", bufs=1, space="PSUM"))

        # --- constants, loaded once; all on the scalar HWDGE queue so the
        # sync queue is dedicated to x prefetch ---
        wq_sb = const.tile([P, CT, D], F8)
        nc.scalar.dma_start(wq_sb[:], wqT.rearrange("(t p) d -> p t d", p=P))
        wk_sb = const.tile([P, CT, D], F8)
        wv_sb = const.tile([P, CT, D], F8)
        wv16_sb = const.tile([P, CT, D], F16)
        bq4_sb = const.tile([P, DT], F32)
        nc.scalar.dma_start(bq4_sb[:], bq4[:])
        bq1_sb = const.tile([P, DT], F32)
        nc.scalar.dma_start(bq1_sb[:], bq1[:])
        bk_sb = const.tile([P, DT], F32)
        nc.scalar.dma_start(bk_sb[:], bk128[:])
        bv32_sb = const.tile([P, D], F32)
        nc.scalar.dma_start(bv32_sb[:], bv32[:])
        bv1_sb = const.tile([P, D], F32)
        nc.scalar.dma_start(bv1_sb[:], bv1[:])
        tri_sb = const.tile([P, P], F16)
        nc.scalar.dma_start(tri_sb[:], tri[:])
        ones16_sb = const.tile([P, 1], F16)
        nc.vector.memset(ones16_sb[:], 1.0)
        ones8_sb = const.tile([P, 2, 1], F8)
        nc.vector.memset(ones8_sb[:], VS)

        # PE warmup: dummy matmuls with no input deps keep the PE busy while
        # batch-0 inputs stream in, so the HAM clock-gate is already at
        # 2.4 GHz when the real stream starts.
        warm_sb = const.tile([P, 512], F16)
        nc.vector.memset(warm_sb[:], 0.0)
        for w in range(15):
            wps = proj_ps.tile([P, 512], F32, tag="pp", name=f"warm{w}")
            nc.tensor.matmul(wps[:], warm_sb[:, 0:P], warm_sb[:],
                             start=True, stop=True)

        for b in range(nb):
            # --- X^T [128, ct, 1024l] fp8 + fp16 l<128 copy ---
            xt = xt_pool.tile([P, CT, L], F8, tag="xt", name=f"xt_{b}")
            # halves: the Q/K lc=0 groups only need the first 0.5MB
            nc.sync.dma_start(xt[:, :, 0:512], xtb[b, :, :, 0:512])
            nc.sync.dma_start(xt[:, :, 512:L], xtb[b, :, :, 512:L])
            xt16 = xt_pool.tile([P, CT, P], F16, tag="xt16", name=f"xt16_{b}")
            nc.sync.dma_start(xt16[:], xtb16[b])
            pm_sb = pm_pool.tile([P, LT], F32, name=f"pm_{b}")
            nc.sync.dma_start(pm_sb[:], pmt[b])
            if b == 0:
                # behind batch-0 x in the sync ring: wk/wv don't steal HBM
                # bandwidth from the startup-critical x prefetch
                nc.sync.dma_start(wk_sb[:],
                                  wkT.rearrange("(t p) d -> p t d", p=P))
                nc.sync.dma_start(wv_sb[:],
                                  wvT.rearrange("(t p) d -> p t d", p=P))
                nc.sync.dma_start(wv16_sb[:],
                                  wvT16.rearrange("(t p) d -> p t d", p=P))

            # --- Q^T, K^T: fp8 pair tiles [128, 2(dt parity), 1024l] via
            # DoubleRow over ct pairs; + fp16 copies of the l<128 columns.
            # Q drains on ACT, K on DVE. ---
            qt = [qk_pool.tile([P, 2, L], F8, tag=f"qt{dp}", name=f"qt{dp}_{b}")
                  for dp in range(DTP)]
            kt = [qk_pool.tile([P, 2, L], F8, tag=f"kt{dp}", name=f"kt{dp}_{b}")
                  for dp in range(DTP)]
            qt16 = qk_pool.tile([P, DT, P], F16, tag="qt16", name=f"qt16_{b}")
            kt16 = qk_pool.tile([P, DT, P], F16, tag="kt16", name=f"kt16_{b}")
            for name, w_sb, dst, dst16 in (("q", wq_sb, qt, qt16),
                                           ("k", wk_sb, kt, kt16)):
                # lc-outer always: the lc=0 groups only depend on the first
                # half of xt, which lands 1.4us before the second
                order = [(dt, lc) for lc in range(L // 512)
                         for dt in range(DT)]
                for dt, lc in order:
                    ps = proj_ps.tile([P, 512], F32, tag="pp",
                                      name=f"{name}ps{dt}_{lc}_{b}")
                    for cp in range(CTP):
                        nc.tensor.matmul(
                            ps[:],
                            w_sb[:, 2 * cp:2 * cp + 2, dt * P:(dt + 1) * P],
                            xt[:, 2 * cp:2 * cp + 2, lc * 512:(lc + 1) * 512],
                            start=(cp == 0), stop=(cp == CTP - 1),
                            perf_mode=DR)
                    dslc = dst[dt // 2][:, dt % 2, lc * 512:(lc + 1) * 512]
                    if name == "q":
                        # ACT: func(in*scale + bias); bias host-packed as 4bq
                        nc.scalar.activation(dslc, ps[:], AF.Identity,
                                             bias=bq4_sb[:, dt:dt + 1],
                                             scale=QKS / WS)
                        if lc == 0:
                            nc.scalar.activation(dst16[:, dt, :], ps[:, 0:P],
                                                 AF.Identity,
                                                 bias=bq1_sb[:, dt:dt + 1],
                                                 scale=1.0 / WS)
                    else:
                        # DVE: (in + 128bk) * s
                        nc.vector.tensor_scalar(dslc, ps[:],
                                                bk_sb[:, dt:dt + 1], QKS / WS,
                                                ALU.add, ALU.mult)
                        if lc == 0:
                            nc.vector.tensor_scalar(dst16[:, dt, :], ps[:, 0:P],
                                                    bk_sb[:, dt:dt + 1],
                                                    1.0 / WS,
                                                    ALU.add, ALU.mult)

            # --- V: fp8 pair tiles [128l, 2(j parity), 512d] storing 32V,
            # DoubleRow over ct pairs; + fp16 128V[0:128] for the ib=0 row
            # tile ---
            v8 = [v_pool.tile([P, 2, D], F8, tag=f"v{jp}", name=f"v{jp}_{b}")
                  for jp in range(LTP)]
            for lt in range(LT):
                ps = proj_ps.tile([P, D], F32, tag="pp", name=f"vps{lt}_{b}")
                for cp in range(CTP):
                    nc.tensor.matmul(
                        ps[:],
                        xt[:, 2 * cp:2 * cp + 2, lt * P:(lt + 1) * P],
                        wv_sb[:, 2 * cp:2 * cp + 2, :],
                        start=(cp == 0), stop=(cp == CTP - 1), perf_mode=DR)
                nc.vector.tensor_add(v8[lt // 2][:, lt % 2, :], ps[:],
                                     bv32_sb[:])
            v16 = v_pool.tile([P, D], F16, tag="v16", name=f"v16_{b}")
            ps = proj_ps.tile([P, D], F32, tag="pp", name=f"v16ps_{b}")
            for ct in range(CT):
                nc.tensor.matmul(ps[:], xt16[:, ct, :], wv16_sb[:, ct, :],
                                 start=(ct == 0), stop=(ct == CT - 1))
            nc.vector.tensor_add(v16[:], ps[:], bv1_sb[:])

            # --- S^T tiles + exp -> P^T fp8 pair tiles [128, 2(j parity),
            # 1024i] (causal: only i >= j0; fp8 chunks start at i=128).
            # The (i<128, j<128) corner runs fp16 -> pt16. ---
            pt = [pt_pool.tile([P, 2, L], F8, tag=f"pt{jp}", name=f"pt{jp}_{b}")
                  for jp in range(LTP)]
            pt16 = pt_pool.tile([P, P], F16, tag="pt16", name=f"pt16_{b}")
            # fp16 corner: j<128, i<128
            ps = s_ps.tile([P, P], F32, tag="sp", name=f"s16_{b}")
            for dt in range(DT):
                nc.tensor.matmul(ps[:], kt16[:, dt, :], qt16[:, dt, :],
                                 start=(dt == 0), stop=(dt == DT - 1))
            nc.scalar.activation(pt16[:], ps[:], AF.Exp,
                                 bias=pm_sb[:, 0:1], scale=SCALE)
            nc.gpsimd.tensor_mul(pt16[:], pt16[:], tri_sb[:])
            # fp8 chunks; gap memsets for odd jb>=3 (even-ib pair reads)
            for jb in range(3, LT, 2):
                nc.gpsimd.memset(pt[jb // 2][:, jb % 2, (jb - 1) * P:jb * P],
                                 0.0)
            for jb in range(LT):
                j0 = jb * P
                t = pt[jb // 2]
                ko = jb % 2
                i0 = max(j0, P)
                while i0 < L:
                    n = min((i0 // 512 + 1) * 512, L) - i0
                    ps = s_ps.tile([P, n], F32, tag="sp",
                                   name=f"sps{jb}_{i0}_{b}")
                    for dp in range(DTP):
                        nc.tensor.matmul(ps[:], kt[dp][:, :, j0:j0 + P],
                                         qt[dp][:, :, i0:i0 + n],
                                         start=(dp == 0), stop=(dp == DTP - 1),
                                         perf_mode=DR)
                    nc.scalar.activation(t[:, ko, i0:i0 + n], ps[:], AF.Exp,
                                         bias=pm_sb[:, jb:jb + 1],
                                         scale=EXP_SCALE8)
                    i0 += n
                if jb > 0:
                    # mask the diagonal tile: keep j<=i (GpSimd: SBUF-only op,
                    # keeps DVE for PSUM drains)
                    nc.gpsimd.tensor_mul(t[:, ko, j0:j0 + P],
                                         t[:, ko, j0:j0 + P], tri_sb[:])

            # --- O' = P^T.T @ V (DoubleRow over j pairs; ib=0 fp16),
            # rowsum = P^T.T @ ones; normalize+relu on GpSimd ---
            for ib in range(LT):
                i0 = ib * P
                ops = o_ps.tile([P, D], F32, tag="op", name=f"ops{ib}_{b}")
                rps = r_ps.tile([P, 1], F32, tag="rp", name=f"rps{ib}_{b}")
                if ib == 0:
                    nc.tensor.matmul(ops[:], pt16[:], v16[:],
                                     start=True, stop=True)
                    nc.tensor.matmul(rps[:], pt16[:], ones16_sb[:],
                                     start=True, stop=True)
                else:
                    njp = ib // 2 + 1
                    for jp in range(njp):
                        pT = pt[jp][:, :, i0:i0 + P]
                        nc.tensor.matmul(ops[:], pT, v8[jp][:],
                                         start=(jp == 0), stop=(jp == njp - 1),
                                         perf_mode=DR)
                        # rowsum as two plain-fp8 MMs: their FWL LDWEIGHTS
                        # (~30ns) hide under the o-MM, unlike a DR LDW (162ns)
                        for ko in range(2):
                            nc.tensor.matmul(
                                rps[:], pt[jp][:, ko, i0:i0 + P],
                                ones8_sb[:, ko, :],
                                start=(jp == 0 and ko == 0),
                                stop=(jp == njp - 1 and ko == 1))
                rec = sm_pool.tile([P, 1], F32, tag="rec", name=f"rec{ib}_{b}")
                nc.vector.reciprocal(rec[:], rps[:])
                o_sb = o_pool.tile([P, D], F32, tag="ot", name=f"o{ib}_{b}")
                # relu(O'/rowsum) = Relu(ops * rec) on ACT (per-partition
                # scale); keeps DVE free so the reciprocal drains rps fast
                # (r_ps is single-buffered)
                nc.scalar.activation(o_sb[:], ops[:], AF.Relu,
                                     bias=0.0, scale=rec[:])
                # SWDGE so stores never head-of-line-block the x prefetch;
                # last batch has no prefetch left, so use the faster HWDGE
                if b == nb - 1:
                    nc.sync.dma_start(out[b, i0:i0 + P, :], o_sb[:])
                else:
                    nc.gpsimd.dma_start(out[b, i0:i0 + P, :], o_sb[:])

    nc.compile()
    return nc


def _prep_host(x, Wq, bq, Wk, bk, Wv, bv, mask):
    f8 = ml_dtypes.float8_e4m3  # TRN-matching: max +-240, inf above
    # x transposed to [B, C, L], packed [B, 128, CT, L] (c = 128*ct + ki)
    xT = np.clip(x.astype(np.float32), -240.0, 240.0).transpose(0, 2, 1)
    xp = np.ascontiguousarray(xT.reshape(B, CT, P, L).transpose(0, 2, 1, 3))
    xb8 = xp.astype(f8)
    xb16 = np.ascontiguousarray(xp[:, :, :, 0:P]).astype(np.float16)
    wqT = np.ascontiguousarray(Wq.T * WS).astype(f8)  # [C, D], prescaled
    wkT = np.ascontiguousarray(Wk.T * WS).astype(f8)
    wvT = np.ascontiguousarray(Wv.T * VS).astype(f8)
    wvT16 = np.ascontiguousarray(Wv.T).astype(np.float16)
    bq4a = np.ascontiguousarray(
        (bq * QKS).astype(np.float32).reshape(DT, P).T)  # [P, DT]
    bq1a = np.ascontiguousarray(bq.astype(np.float32).reshape(DT, P).T)
    bk128a = np.ascontiguousarray(
        (bk * WS).astype(np.float32).reshape(DT, P).T)
    bv32a = np.ascontiguousarray(
        np.broadcast_to((bv * VS).astype(np.float32), (P, D)))
    bv1a = np.ascontiguousarray(
        np.broadcast_to(bv.astype(np.float32), (P, D)))
    pm = np.where(mask[:, 0, :] != 0, 0.0, NEG).astype(np.float32)  # [B, L]
    pmt = np.ascontiguousarray(
        pm.reshape(B, LT, P).transpose(0, 2, 1))  # [B, P, LT]
    tri = (np.arange(P)[:, None] <= np.arange(P)[None, :]).astype(np.float16)
    return (xb8, xb16, wqT, wkT, wvT, wvT16, bq4a, bq1a, bk128a, bv32a,
            bv1a, pmt, tri)


_NC_CACHE = {}


def kernel(x, Wq, bq, Wk, bk, Wv, bv, mask):
    x = np.asarray(x)
    Wq, bq = np.asarray(Wq), np.asarray(bq)
    Wk, bk = np.asarray(Wk), np.asarray(bk)
    Wv, bv = np.asarray(Wv), np.asarray(bv)
    mask = np.asarray(mask)

    (xb8, xb16, wqT, wkT, wvT, wvT16, bq4a, bq1a, bk128a, bv32a, bv1a,
     pmt, tri) = _prep_host(x, Wq, bq, Wk, bk, Wv, bv, mask)

    if "nc" not in _NC_CACHE:
        _NC_CACHE["nc"] = build_program(NB)
    nc = _NC_CACHE["nc"]

    in_maps = []
    for c in range(N_CORES):
        s = slice(c * NB, (c + 1) * NB)
        in_maps.append({
            "xtb": np.ascontiguousarray(xb8[s]),
            "xtb16": np.ascontiguousarray(xb16[s]),
            "wqT": wqT, "wkT": wkT, "wvT": wvT, "wvT16": wvT16,
            "bq4": bq4a, "bq1": bq1a, "bk128": bk128a,
            "bv32": bv32a, "bv1": bv1a,
            "pmt": np.ascontiguousarray(pmt[s]),
            "tri": tri,
        })

    res = bass_utils.run_bass_kernel_spmd(
        nc, in_maps, core_ids=list(range(N_CORES)),
        trace=bool(int(os.environ.get("KERNEL_TRACE", "0"))),
    )
    if os.environ.get("KERNEL_RESULT_HOOK"):
        _NC_CACHE["last_result"] = res

    return np.concatenate([res.results[c]["out"] for c in range(N_CORES)],
                          axis=0)


# revision 11
# speedup vs baseline: 1.1327x; 1.1327x over previous
"""Causal attention (QKV proj + softmax + PV + ReLU) on 8 trn2 NeuronCores.

Sharding: data-parallel over batch B=32 -> 4 batches per core; projection
weights replicated.

Dtype strategy (error budget is max-abs-err / global absmax < 2e-2; host
numpy simulation of the exact quantization pipeline predicts 1.58e-2):
  * All projections and most of the attention run as fp8(e4m3) DoubleRow
    matmuls: the PE streams 1 output column/cycle regardless of dtype,
    but DoubleRow packs TWO 128-row contraction blocks per matmul
    (APs [128, 2, cols]), halving matmul count for the C=1024 (proj) and
    D=512 (S) contractions.
  * The output's first 128 rows attend to <=128 keys, so softmax weight
    concentrates there and fp8 noise on P/V/logits would blow the max-err
    budget. Those rows get a small fp16 path: fp16 V[0:128] (from an fp16
    copy of x[:, :, :128] + fp16 wv), an fp16 (i<128, j<128) logit block
    (from fp16-stored q/k), fp16 P, fp16 PV for the ib=0 row tile. For
    i>=128 softmax weight is spread (<~3% each), so fp8-quantized q/k
    storage, P, and V noise all average out.
  * fp8 scaling: W rows ~U(-1/32,1/32) sit below e4m3's min normal 2^-6,
    so wq/wk are host-prescaled x128 (and wv x32). Q/K drain at x1/32
    (fp8 stores 4Q; 16x folds into the exp scale) and x1/128 (fp16 corner
    copies). V drains at +32bv into fp8 (32V); the 32 folds into the
    fp8 rowsum via ones8=32 (fp16 path: plain V, ones16=1).

Per core, per batch:
  Q^T,K^T[d,l]: 4 DoubleRow MMs per [d-block, l-chunk]; Q drains on ACT
      (activation Identity, per-partition bias), K on DVE - splits the
      drain load across engines. Stored as fp8 pair tiles [128,2,L]
      (dt pairs, ready as DoubleRow operands) + tiny fp16 l<128 copies.
  V[l,d]: 4 DoubleRow MMs per l-tile -> fp8 pair tiles [128,2,512]
      (j-parity pairs for the PV DoubleRow contraction); plus fp16
      V[0:128] from the fp16 x/wv copies.
  S^T[j,i]: 2 DoubleRow MMs per causal chunk (d pairs); the (i<128,j<128)
      corner chunk runs fp16. exp on ACT (scale SCALE/16 fp8 / SCALE
      fp16, per-partition padmask bias) drains P^T straight to fp8 pair
      tiles [128,2,L] (j-parity); diagonal tiles masked by DVE multiply
      with tri; gap regions of odd-j tiles memset to 0 so even-ib
      DoubleRow reads see zeros.
  O' = P^T.T @ V as DoubleRow over j-pairs (ib=0: fp16); rowsum
      piggybacks on the stationary (moving ones8 pair / ones16).
  out = Relu(O' * (1/rowsum)): reciprocal on DVE, normalize+relu on
      GpSimd (keeps DVE off the PV critical path), DMA out.

DMA-queue/warmup notes: x prefetch on nc.sync, const loads on nc.scalar,
stores on nc.gpsimd (SWDGE) so they never head-of-line-block the
prefetch; dummy warmup matmuls pre-warm the PE HAM clock-gate to 2.4 GHz
while batch-0 inputs stream in; wk/wv loads deferred behind batch-0 x.
"""

import os
from contextlib import ExitStack

import numpy as np
import ml_dtypes

import concourse.tile as tile
from concourse import bacc, mybir
from concourse import bass_utils

F32 = mybir.dt.float32
F16 = mybir.dt.float16
F8 = mybir.dt.float8e4
DR = mybir.MatmulPerfMode.DoubleRow
AF = mybir.ActivationFunctionType
ALU = mybir.AluOpType

N_CORES = 8
B = 32
L = 1024
C = 1024  # d_model
D = 512
P = 128
NB = B // N_CORES  # batches per core
CT = C // P  # 8 contraction tiles
CTP = CT // 2  # 4 DoubleRow contraction pairs
DT = D // P  # 4 d tiles
DTP = DT // 2  # 2 DoubleRow d pairs
LT = L // P  # 8 l/j/i tiles
LTP = LT // 2  # 4 j-pair tiles
SCALE = float(D) ** -0.5
NEG = -30000.0
WS = 128.0  # host prescale on wq/wk
VS = 32.0  # host prescale on wv (fp8 V stored as 32V)
QKS = 4.0  # fp8 q/k stored as 4Q
EXP_SCALE8 = SCALE / (QKS * QKS)


def build_program(nb: int = NB):
    """Build the per-core Bass program for nb batches."""
    nc = bacc.Bacc("TRN2", target_bir_lowering=False, debug=False,
                   num_devices=N_CORES)

    xtb = nc.dram_tensor("xtb", [nb, P, CT, L], F8, kind="ExternalInput").ap()
    xtb16 = nc.dram_tensor("xtb16", [nb, P, CT, P], F16,
                           kind="ExternalInput").ap()
    wqT = nc.dram_tensor("wqT", [C, D], F8, kind="ExternalInput").ap()
    wkT = nc.dram_tensor("wkT", [C, D], F8, kind="ExternalInput").ap()
    wvT = nc.dram_tensor("wvT", [C, D], F8, kind="ExternalInput").ap()
    wvT16 = nc.dram_tensor("wvT16", [C, D], F16, kind="ExternalInput").ap()
    bq4 = nc.dram_tensor("bq4", [P, DT], F32, kind="ExternalInput").ap()
    bq1 = nc.dram_tensor("bq1", [P, DT], F32, kind="ExternalInput").ap()
    bk128 = nc.dram_tensor("bk128", [P, DT], F32, kind="ExternalInput").ap()
    bv32 = nc.dram_tensor("bv32", [P, D], F32, kind="ExternalInput").ap()
    bv1 = nc.dram_tensor("bv1", [P, D], F32, kind="ExternalInput").ap()
    pmt = nc.dram_tensor("pmt", [nb, P, LT], F32, kind="ExternalInput").ap()
    tri = nc.dram_tensor("tri", [P, P], F16, kind="ExternalInput").ap()
    out = nc.dram_tensor("out", [nb, L, D], F32, kind="ExternalOutput").ap()

    with tile.TileContext(nc) as tc, ExitStack() as ctx:
        const = ctx.enter_context(tc.tile_pool(name="const", bufs=1))
        xt_pool = ctx.enter_context(tc.tile_pool(name="xt", bufs=3))
        qk_pool = ctx.enter_context(tc.tile_pool(name="qk", bufs=2))
        v_pool = ctx.enter_context(tc.tile_pool(name="v", bufs=2))
        pt_pool = ctx.enter_context(tc.tile_pool(name="pt", bufs=2))
        o_pool = ctx.enter_context(tc.tile_pool(name="o", bufs=3))
        sm_pool = ctx.enter_context(tc.tile_pool(name="sm", bufs=4))
        pm_pool = ctx.enter_context(tc.tile_pool(name="pm", bufs=2))
        proj_ps = ctx.enter_context(tc.tile_pool(name="pps", bufs=3, space="PSUM"))
        s_ps = ctx.enter_context(tc.tile_pool(name="sps", bufs=2, space="PSUM"))
        o_ps = ctx.enter_context(tc.tile_pool(name="ops", bufs=2, space="PSUM"))
        r_ps = ctx.enter_context(tc.tile_pool(name="rps", bufs=1, space="PSUM"))

        # --- constants, loaded once; all on the scalar HWDGE queue so the
        # sync queue is dedicated to x prefetch ---
        wq_sb = const.tile([P, CT, D], F8)
        nc.scalar.dma_start(wq_sb[:], wqT.rearrange("(t p) d -> p t d", p=P))
        wk_sb = const.tile([P, CT, D], F8)
        wv_sb = const.tile([P, CT, D], F8)
        wv16_sb = const.tile([P, CT, D], F16)
        bq4_sb = const.tile([P, DT], F32)
        nc.scalar.dma_start(bq4_sb[:], bq4[:])
        bq1_sb = const.tile([P, DT], F32)
        nc.scalar.dma_start(bq1_sb[:], bq1[:])
        bk_sb = const.tile([P, DT], F32)
        nc.scalar.dma_start(bk_sb[:], bk128[:])
        bv32_sb = const.tile([P, D], F32)
        nc.scalar.dma_start(bv32_sb[:], bv32[:])
        bv1_sb = const.tile([P, D], F32)
        nc.scalar.dma_start(bv1_sb[:], bv1[:])
        tri_sb = const.tile([P, P], F16)
        nc.scalar.dma_start(tri_sb[:], tri[:])
        ones16_sb = const.tile([P, 1], F16)
        nc.vector.memset(ones16_sb[:], 1.0)
        ones8_sb = const.tile([P, 2, 1], F8)
        nc.vector.memset(ones8_sb[:], VS)

        # PE warmup: dummy matmuls with no input deps keep the PE busy while
        # batch-0 inputs stream in, so the HAM clock-gate is already at
        # 2.4 GHz when the real stream starts.
        warm_sb = const.tile([P, 512], F16)
        nc.vector.memset(warm_sb[:], 0.0)
        for w in range(15):
            wps = proj_ps.tile([P, 512], F32, tag="pp", name=f"warm{w}")
            nc.tensor.matmul(wps[:], warm_sb[:, 0:P], warm_sb[:],
                             start=True, stop=True)

        for b in range(nb):
            # --- X^T [128, ct, 1024l] fp8 + fp16 l<128 copy ---
            xt = xt_pool.tile([P, CT, L], F8, tag="xt", name=f"xt_{b}")
            if b == 0:
                # first batch: l<512 halves first so the Q lc=0 groups can
                # start after 0.5MB instead of 1MB
                nc.sync.dma_start(xt[:, :, 0:512], xtb[b, :, :, 0:512])
                nc.sync.dma_start(xt[:, :, 512:L], xtb[b, :, :, 512:L])
            else:
                nc.sync.dma_start(xt[:], xtb[b])
            xt16 = xt_pool.tile([P, CT, P], F16, tag="xt16", name=f"xt16_{b}")
            nc.sync.dma_start(xt16[:], xtb16[b])
            pm_sb = pm_pool.tile([P, LT], F32, name=f"pm_{b}")
            nc.sync.dma_start(pm_sb[:], pmt[b])
            if b == 0:
                # behind batch-0 x in the sync ring: wk/wv don't steal HBM
                # bandwidth from the startup-critical x prefetch
                nc.sync.dma_start(wk_sb[:],
                                  wkT.rearrange("(t p) d -> p t d", p=P))
                nc.sync.dma_start(wv_sb[:],
                                  wvT.rearrange("(t p) d -> p t d", p=P))
                nc.sync.dma_start(wv16_sb[:],
                                  wvT16.rearrange("(t p) d -> p t d", p=P))

            # --- Q^T, K^T: fp8 pair tiles [128, 2(dt parity), 1024l] via
            # DoubleRow over ct pairs; + fp16 copies of the l<128 columns.
            # Q drains on ACT, K on DVE. ---
            qt = [qk_pool.tile([P, 2, L], F8, tag=f"qt{dp}", name=f"qt{dp}_{b}")
                  for dp in range(DTP)]
            kt = [qk_pool.tile([P, 2, L], F8, tag=f"kt{dp}", name=f"kt{dp}_{b}")
                  for dp in range(DTP)]
            qt16 = qk_pool.tile([P, DT, P], F16, tag="qt16", name=f"qt16_{b}")
            kt16 = qk_pool.tile([P, DT, P], F16, tag="kt16", name=f"kt16_{b}")
            for name, w_sb, dst, dst16 in (("q", wq_sb, qt, qt16),
                                           ("k", wk_sb, kt, kt16)):
                if b == 0 and name == "q":
                    # lc-outer so all lc=0 groups run on the early halves
                    order = [(dt, lc) for lc in range(L // 512)
                             for dt in range(DT)]
                else:
                    order = [(dt, lc) for dt in range(DT)
                             for lc in range(L // 512)]
                for dt, lc in order:
                    ps = proj_ps.tile([P, 512], F32, tag="pp",
                                      name=f"{name}ps{dt}_{lc}_{b}")
                    for cp in range(CTP):
                        nc.tensor.matmul(
                            ps[:],
                            w_sb[:, 2 * cp:2 * cp + 2, dt * P:(dt + 1) * P],
                            xt[:, 2 * cp:2 * cp + 2, lc * 512:(lc + 1) * 512],
                            start=(cp == 0), stop=(cp == CTP - 1),
                            perf_mode=DR)
                    dslc = dst[dt // 2][:, dt % 2, lc * 512:(lc + 1) * 512]
                    if name == "q":
                        # ACT: func(in*scale + bias); bias host-packed as 4bq
                        nc.scalar.activation(dslc, ps[:], AF.Identity,
                                             bias=bq4_sb[:, dt:dt + 1],
                                             scale=QKS / WS)
                        if lc == 0:
                            nc.scalar.activation(dst16[:, dt, :], ps[:, 0:P],
                                                 AF.Identity,
                                                 bias=bq1_sb[:, dt:dt + 1],
                                                 scale=1.0 / WS)
                    else:
                        # DVE: (in + 128bk) * s
                        nc.vector.tensor_scalar(dslc, ps[:],
                                                bk_sb[:, dt:dt + 1], QKS / WS,
                                                ALU.add, ALU.mult)
                        if lc == 0:
                            nc.vector.tensor_scalar(dst16[:, dt, :], ps[:, 0:P],
                                                    bk_sb[:, dt:dt + 1],
                                                    1.0 / WS,
                                                    ALU.add, ALU.mult)

            # --- V: fp8 pair tiles [128l, 2(j parity), 512d] storing 32V,
            # DoubleRow over ct pairs; + fp16 128V[0:128] for the ib=0 row
            # tile ---
            v8 = [v_pool.tile([P, 2, D], F8, tag=f"v{jp}", name=f"v{jp}_{b}")
                  for jp in range(LTP)]
            for lt in range(LT):
                ps = proj_ps.tile([P, D], F32, tag="pp", name=f"vps{lt}_{b}")
                for cp in range(CTP):
                    nc.tensor.matmul(
                        ps[:],
                        xt[:, 2 * cp:2 * cp + 2, lt * P:(lt + 1) * P],
                        wv_sb[:, 2 * cp:2 * cp + 2, :],
                        start=(cp == 0), stop=(cp == CTP - 1), perf_mode=DR)
                nc.vector.tensor_add(v8[lt // 2][:, lt % 2, :], ps[:],
                                     bv32_sb[:])
            v16 = v_pool.tile([P, D], F16, tag="v16", name=f"v16_{b}")
            ps = proj_ps.tile([P, D], F32, tag="pp", name=f"v16ps_{b}")
            for ct in range(CT):
                nc.tensor.matmul(ps[:], xt16[:, ct, :], wv16_sb[:, ct, :],
                                 start=(ct == 0), stop=(ct == CT - 1))
            nc.vector.tensor_add(v16[:], ps[:], bv1_sb[:])

            # --- S^T tiles + exp -> P^T fp8 pair tiles [128, 2(j parity),
            # 1024i] (causal: only i >= j0; fp8 chunks start at i=128).
            # The (i<128, j<128) corner runs fp16 -> pt16. ---
            pt = [pt_pool.tile([P, 2, L], F8, tag=f"pt{jp}", name=f"pt{jp}_{b}")
                  for jp in range(LTP)]
            pt16 = pt_pool.tile([P, P], F16, tag="pt16", name=f"pt16_{b}")
            # fp16 corner: j<128, i<128
            ps = s_ps.tile([P, P], F32, tag="sp", name=f"s16_{b}")
            for dt in range(DT):
                nc.tensor.matmul(ps[:], kt16[:, dt, :], qt16[:, dt, :],
                                 start=(dt == 0), stop=(dt == DT - 1))
            nc.scalar.activation(pt16[:], ps[:], AF.Exp,
                                 bias=pm_sb[:, 0:1], scale=SCALE)
            nc.gpsimd.tensor_mul(pt16[:], pt16[:], tri_sb[:])
            # fp8 chunks; gap memsets for odd jb>=3 (even-ib pair reads)
            for jb in range(3, LT, 2):
                nc.gpsimd.memset(pt[jb // 2][:, jb % 2, (jb - 1) * P:jb * P],
                                 0.0)
            for jb in range(LT):
                j0 = jb * P
                t = pt[jb // 2]
                ko = jb % 2
                i0 = max(j0, P)
                while i0 < L:
                    n = min((i0 // 512 + 1) * 512, L) - i0
                    ps = s_ps.tile([P, n], F32, tag="sp",
                                   name=f"sps{jb}_{i0}_{b}")
                    for dp in range(DTP):
                        nc.tensor.matmul(ps[:], kt[dp][:, :, j0:j0 + P],
                                         qt[dp][:, :, i0:i0 + n],
                                         start=(dp == 0), stop=(dp == DTP - 1),
                                         perf_mode=DR)
                    nc.scalar.activation(t[:, ko, i0:i0 + n], ps[:], AF.Exp,
                                         bias=pm_sb[:, jb:jb + 1],
                                         scale=EXP_SCALE8)
                    i0 += n
                if jb > 0:
                    # mask the diagonal tile: keep j<=i (GpSimd: SBUF-only op,
                    # keeps DVE for PSUM drains)
                    nc.gpsimd.tensor_mul(t[:, ko, j0:j0 + P],
                                         t[:, ko, j0:j0 + P], tri_sb[:])

            # --- O' = P^T.T @ V (DoubleRow over j pairs; ib=0 fp16),
            # rowsum = P^T.T @ ones; normalize+relu on GpSimd ---
            for ib in range(LT):
                i0 = ib * P
                ops = o_ps.tile([P, D], F32, tag="op", name=f"ops{ib}_{b}")
                rps = r_ps.tile([P, 1], F32, tag="rp", name=f"rps{ib}_{b}")
                if ib == 0:
                    nc.tensor.matmul(ops[:], pt16[:], v16[:],
                                     start=True, stop=True)
                    nc.tensor.matmul(rps[:], pt16[:], ones16_sb[:],
                                     start=True, stop=True)
                else:
                    njp = ib // 2 + 1
                    for jp in range(njp):
                        pT = pt[jp][:, :, i0:i0 + P]
                        nc.tensor.matmul(ops[:], pT, v8[jp][:],
                                         start=(jp == 0), stop=(jp == njp - 1),
                                         perf_mode=DR)
                        # rowsum as two plain-fp8 MMs: their FWL LDWEIGHTS
                        # (~30ns) hide under the o-MM, unlike a DR LDW (162ns)
                        for ko in range(2):
                            nc.tensor.matmul(
                                rps[:], pt[jp][:, ko, i0:i0 + P],
                                ones8_sb[:, ko, :],
                                start=(jp == 0 and ko == 0),
                                stop=(jp == njp - 1 and ko == 1))
                rec = sm_pool.tile([P, 1], F32, tag="rec", name=f"rec{ib}_{b}")
                nc.vector.reciprocal(rec[:], rps[:])
                o_sb = o_pool.tile([P, D], F32, tag="ot", name=f"o{ib}_{b}")
                # relu(O'/rowsum) = Relu(ops * rec) on ACT (per-partition
                # scale); keeps DVE free so the reciprocal drains rps fast
                # (r_ps is single-buffered)
                nc.scalar.activation(o_sb[:], ops[:], AF.Relu,
                                     bias=0.0, scale=rec[:])
                # SWDGE so stores never head-of-line-block the x prefetch;
                # last batch has no prefetch left, so use the faster HWDGE
                if b == nb - 1:
                    nc.sync.dma_start(out[b, i0:i0 + P, :], o_sb[:])
                else:
                    nc.gpsimd.dma_start(out[b, i0:i0 + P, :], o_sb[:])

    nc.compile()
    return nc


def _prep_host(x, Wq, bq, Wk, bk, Wv, bv, mask):
    f8 = ml_dtypes.float8_e4m3  # TRN-matching: max +-240, inf above
    # x transposed to [B, C, L], packed [B, 128, CT, L] (c = 128*ct + ki)
    xT = np.clip(x.astype(np.float32), -240.0, 240.0).transpose(0, 2, 1)
    xp = np.ascontiguousarray(xT.reshape(B, CT, P, L).transpose(0, 2, 1, 3))
    xb8 = xp.astype(f8)
    xb16 = np.ascontiguousarray(xp[:, :, :, 0:P]).astype(np.float16)
    wqT = np.ascontiguousarray(Wq.T * WS).astype(f8)  # [C, D], prescaled
    wkT = np.ascontiguousarray(Wk.T * WS).astype(f8)
    wvT = np.ascontiguousarray(Wv.T * VS).astype(f8)
    wvT16 = np.ascontiguousarray(Wv.T).astype(np.float16)
    bq4a = np.ascontiguousarray(
        (bq * QKS).astype(np.float32).reshape(DT, P).T)  # [P, DT]
    bq1a = np.ascontiguousarray(bq.astype(np.float32).reshape(DT, P).T)
    bk128a = np.ascontiguousarray(
        (bk * WS).astype(np.float32).reshape(DT, P).T)
    bv32a = np.ascontiguousarray(
        np.broadcast_to((bv * VS).astype(np.float32), (P, D)))
    bv1a = np.ascontiguousarray(
        np.broadcast_to(bv.astype(np.float32), (P, D)))
    pm = np.where(mask[:, 0, :] != 0, 0.0, NEG).astype(np.float32)  # [B, L]
    pmt = np.ascontiguousarray(
        pm.reshape(B, LT, P).transpose(0, 2, 1))  # [B, P, LT]
    tri = (np.arange(P)[:, None] <= np.arange(P)[None, :]).astype(np.float16)
    return (xb8, xb16, wqT, wkT, wvT, wvT16, bq4a, bq1a, bk128a, bv32a,
            bv1a, pmt, tri)


_NC_CACHE = {}


def kernel(x, Wq, bq, Wk, bk, Wv, bv, mask):
    x = np.asarray(x)
    Wq, bq = np.asarray(Wq), np.asarray(bq)
    Wk, bk = np.asarray(Wk), np.asarray(bk)
    Wv, bv = np.asarray(Wv), np.asarray(bv)
    mask = np.asarray(mask)

    (xb8, xb16, wqT, wkT, wvT, wvT16, bq4a, bq1a, bk128a, bv32a, bv1a,
     pmt, tri) = _prep_host(x, Wq, bq, Wk, bk, Wv, bv, mask)

    if "nc" not in _NC_CACHE:
        _NC_CACHE["nc"] = build_program(NB)
    nc = _NC_CACHE["nc"]

    in_maps = []
    for c in range(N_CORES):
        s = slice(c * NB, (c + 1) * NB)
        in_maps.append({
            "xtb": np.ascontiguousarray(xb8[s]),
            "xtb16": np.ascontiguousarray(xb16[s]),
            "wqT": wqT, "wkT": wkT, "wvT": wvT, "wvT16": wvT16,
            "bq4": bq4a, "bq1": bq1a, "bk128": bk128a,
            "bv32": bv32a, "bv1": bv1a,
            "pmt": np.ascontiguousarray(pmt[s]),
            "tri": tri,
        })

    res = bass_utils.run_bass_kernel_spmd(
        nc, in_maps, core_ids=list(range(N_CORES)),
        trace=bool(int(os.environ.get("KERNEL_TRACE", "0"))),
    )
    if os.environ.get("KERNEL_RESULT_HOOK"):
        _NC_CACHE["last_result"] = res

    return np.concatenate([res.results[c]["out"] for c in range(N_CORES)],
                          axis=0)
